# revision 16
# baseline (speedup 1.0000x reference)
"""Trainium2 Bass kernel for nn_CAttentionLegacy (channel attention).

Per-batch-element pipeline (1 batch element per NeuronCore, 8 cores), all
matmul operands bf16 (fp32 PSUM accumulation):

  Pass 1 (qk conv): the reference's 1x1 qkv conv + dense 3x3 conv compose
     into a single 3x3 conv with host-precomputed weights
     W_f[o,i,dy,dx] = sum_m W_dw[o,m,dy,dx] * W_qkv[m,i].  Only the q,k
     output channels (384) are produced.  q,k come out transposed
     ([spatial, channel], x-patch as stationary operand) feeding
     Gram-matrix accumulation G = q^T k and sum-of-squares (for the L2
     norms) without any transposes.  The 64 leftover input channels
     (192-128) are "K-packed": two 3x3 taps of the lo channels are stacked
     into one 128-partition stationary using column-/row-shifted copies of
     the lo x-window, so 17 mixed-K matmuls become 14 full-K ones.
  Attention: attn = softmax(G * rq[c] * rk[d] * temp[head]) per 32x32 head
     block; C^T = A^T @ Wp^T (proj folded in, tiny).
  Compose: Wy^T[i,o] = sum_d Wv[d,i,t] C^T[d,o] on-device (36 tiny
     matmuls) — the attention-weighted combination of v fuses INTO the
     v conv weights.
  Pass 2 (y conv): y = Wy * x as a single 3x3 conv streamed over x again
     (same K-packing), writing the final output directly.  No v spill, no
     second attention pass.
"""
import sys
sys.path.insert(0, '/opt/trn_rl_repo')

import time
import numpy as np
import jax

import concourse.bass as bass
import concourse.tile as tile
from concourse import mybir, bass2jax

HEADS = 6
DIM = 192
B = 8
HW = 128 * 128
PADW = 130  # padded row length
XCOLS = PADW * PADW + 4  # +4 pad so the col-shifted lo-window load stays in bounds
F32 = mybir.dt.float32
F32R = mybir.dt.float32r
BF16 = mybir.dt.bfloat16
AX = mybir.AxisListType
AF = mybir.ActivationFunctionType

# lo-channel tap packing: taps t = 3*dy+dx.
# A-buffer pairs (same row, +2 col shift): (dy,0)+(dy,2)
PAIRS_A = [(0, 2), (3, 5), (6, 8)]
# B-buffer pair (+2 row shift, same col): (0,1)+(2,1)
PAIR_B = (1, 7)
TAP_SINGLE = 4  # (1,1), K=64 via lo half of the A buffer


def replace_range_clears(nc):
    """The For_i back-edge resets loop semaphores with an InstISA
    EVENT_SEMAPHORE_RANGE_CLEAR, which this walrus rejects ('ISA wrong
    length').  Replace each with per-semaphore EventSemaphore writes."""
    import re
    n = 0
    for f in nc.m.functions:
        for bb in f.blocks:
            insts = bb.instructions
            new = []
            changed = False
            for inst in insts:
                if (type(inst).__name__ == "InstISA"
                        and getattr(inst, "isa_opcode", None) == 176):
                    m = re.search(r"range_first=(\d+) range_last=(\d+)",
                                  inst.concise())
                    lo, hi = int(m.group(1)), int(m.group(2))
                    si = inst.sync_info
                    waits = list(si.on_wait) if si is not None else []
                    upds = list(si.on_update) if si is not None else []
                    ids = list(range(lo, hi + 1))
                    for k, sid in enumerate(ids):
                        n += 1
                        ev = mybir.InstEventSemaphore(
                            name=f"rangeclr-{n}", ins=[], outs=[])
                        ev.engine = inst.engine
                        ow = waits if k == 0 else []
                        ou = [mybir.SyncUpdate(
                            sync_type="semaphore", id=sid,
                            update_mode="sem-wr-imm", update_value=0)]
                        if k == len(ids) - 1:
                            ou = ou + upds
                        ev.sync_info = mybir.SyncInfo(on_wait=ow, on_update=ou)
                        new.append(ev)
                    changed = True
                else:
                    new.append(inst)
            if changed:
                insts[:] = new
    return n


def split_multi_waits(nc):
    """This walrus build allows at most ONE sem wait per instruction
    ('Too many sync wait commands').  Hoist extra waits onto same-engine
    nops inserted immediately before the instruction."""
    ctr = 0
    for f in nc.m.functions:
        for bb in f.blocks:
            insts = bb.instructions
            new = []
            changed = False
            for inst in insts:
                si = inst.sync_info
                if si is not None and si.on_wait and len(si.on_wait) > 1:
                    waits = list(si.on_wait)
                    for w in waits[:-1]:
                        ctr += 1
                        nop = mybir.InstNoOp(name=f"splitw-{ctr}", ins=[], outs=[])
                        nop.engine = inst.engine
                        nop.sync_info = mybir.SyncInfo(on_wait=[w], on_update=[])
                        new.append(nop)
                    inst.sync_info = mybir.SyncInfo(
                        on_wait=[waits[-1]], on_update=list(si.on_update))
                    changed = True
                new.append(inst)
            if changed:
                insts[:] = new
    return ctr


class TC(tile.TileContext):
    def __exit__(self, *a):
        r = super().__exit__(*a)
        if a[0] is None:
            replace_range_clears(self.nc)
            split_multi_waits(self.nc)
        return r


def build_nc(R=1, ablate=frozenset()):
    ab = set(ablate)
    nc = bass.Bass("TRN2", target_bir_lowering=False, debug=False)

    xp_d = nc.dram_tensor("xp", [DIM, XCOLS], BF16, kind="ExternalInput")
    # qk conv weights: [ci_hi, t*384 + co] / packed lo [ci-pair, slot*384 + co]
    wqkh_d = nc.dram_tensor("wqk_hi", [128, 9 * 384], BF16, kind="ExternalInput")
    wqkl_d = nc.dram_tensor("wqk_lop", [128, 5 * 384], BF16, kind="ExternalInput")
    # v conv weights for the compose step: [d, t*192 + i]
    wvh_d = nc.dram_tensor("wv_hi", [128, 9 * 192], BF16, kind="ExternalInput")
    wvl_d = nc.dram_tensor("wv_lo", [64, 9 * 192], BF16, kind="ExternalInput")
    # packed-lo compose stationaries: [d, slot*128 + (i_lo(t1) | i_lo(t2))]
    wvhp_d = nc.dram_tensor("wv_hi_p", [128, 4 * 128], BF16, kind="ExternalInput")
    wvlp_d = nc.dram_tensor("wv_lo_p", [64, 4 * 128], BF16, kind="ExternalInput")
    # consts_hi: cols 0:192 WpT rows 0:128 | col 192 ones
    ch_d = nc.dram_tensor("consts_hi", [128, 193], BF16, kind="ExternalInput")
    cl_d = nc.dram_tensor("consts_lo", [64, 192], BF16, kind="ExternalInput")
    # row consts (f32r-bitcast packing): cols 0:192 temp (fp32 bits)
    cr_d = nc.dram_tensor("consts_row", [1, 320], F32R, kind="ExternalInput")
    # bf16 row of ones (stationary for the norm-broadcast matmul)
    or_d = nc.dram_tensor("ones_row", [1, 128], BF16, kind="ExternalInput")
    az_d = nc.dram_tensor("azero", [128, 192], BF16, kind="ExternalInput")
    # y stored transposed: [spatial, ch] (host transposes back)
    y_d = nc.dram_tensor("yt", [HW, DIM], F32, kind="ExternalOutput")

    with TC(nc) as tc:
        import contextlib
        stk = contextlib.ExitStack()
        with stk:
            small = stk.enter_context(tc.tile_pool(name="small", bufs=1))
            wpool = stk.enter_context(tc.tile_pool(name="wpool", bufs=1))

            wqk_hi = wpool.tile([128, 9 * 384], BF16, name="wqk_hi")
            wqk_lop = wpool.tile([128, 5 * 384], BF16, name="wqk_lop")
            wv_hi = wpool.tile([128, 9 * 192], BF16, name="wv_hi")
            wv_lo = wpool.tile([64, 9 * 192], BF16, name="wv_lo")
            wv_hi_p = wpool.tile([128, 4 * 128], BF16, name="wv_hi_p")
            wv_lo_p = wpool.tile([64, 4 * 128], BF16, name="wv_lo_p")
            consts_hi = small.tile([128, 193], BF16, name="consts_hi")
            consts_lo = small.tile([64, 192], BF16, name="consts_lo")
            consts_row = small.tile([1, 320], F32R, name="consts_row")
            ones_row = small.tile([1, 128], BF16, name="ones_row")

            def body(it):
                nc.sync.dma_start(wqk_hi[:], wqkh_d.ap())
                nc.sync.dma_start(wqk_lop[:], wqkl_d.ap())
                nc.sync.dma_start(wv_hi[:], wvh_d.ap())
                nc.sync.dma_start(wv_lo[:], wvl_d.ap())
                nc.sync.dma_start(wv_hi_p[:], wvhp_d.ap())
                nc.sync.dma_start(wv_lo_p[:], wvlp_d.ap())
                nc.sync.dma_start(consts_hi[:], ch_d.ap())
                nc.sync.dma_start(consts_lo[:], cl_d.ap())
                nc.sync.dma_start(consts_row[:], cr_d.ap())
                nc.sync.dma_start(ones_row[:], or_d.ap())
                wpt_hi = consts_hi[:, 0:192]
                wpt_lo = consts_lo[:, 0:192]
                ones_c = consts_hi[:, 192:193]          # [128,1] bf16
                temp_f = consts_row[:, 0:192].bitcast(F32)   # [1,192] fp32
                ones_r = ones_row[:]                    # [1,128] bf16

                with contextlib.ExitStack() as conv_stk:
                    win = conv_stk.enter_context(tc.tile_pool(name="win", bufs=3))
                    qkp = conv_stk.enter_context(tc.tile_pool(name="qkp", bufs=3))
                    sqp_pool = conv_stk.enter_context(tc.tile_pool(name="sqp", bufs=3))
                    att = conv_stk.enter_context(tc.tile_pool(name="att", bufs=1))
                    gps_stk = conv_stk.enter_context(contextlib.ExitStack())
                    gps = gps_stk.enter_context(
                        tc.tile_pool(name="gps", bufs=1, space="PSUM"))
                    mm_stk = contextlib.ExitStack()
                    cps = mm_stk.enter_context(
                        tc.tile_pool(name="cps", bufs=3, space="PSUM"))

                    g_hi = gps.tile([128, 192], F32, name="g_hi")
                    g_lo = gps.tile([64, 192], F32, name="g_lo")
                    ssq_ps = gps.tile([1, 384], F32, name="ssq_ps")

                    # deferred work carried between rows: (qk_sb, y) pairs
                    pend_g = []
                    pend_ssq = []

                    def emit_g(qk_sb, y):
                        nc.tensor.matmul(g_hi[:], qk_sb[:, 0:128],
                                         qk_sb[:, 192:384],
                                         start=(y == 0), stop=(y == 127))
                        nc.tensor.matmul(g_lo[:], qk_sb[:, 128:192],
                                         qk_sb[:, 192:384],
                                         start=(y == 0), stop=(y == 127))

                    def emit_ssq(sq_sb, y):
                        nc.tensor.matmul(ssq_ps[:], ones_c, sq_sb[:],
                                         start=(y == 0), stop=(y == 127))

                    def load_windows(g):
                        """DMA the x window tiles for 4-row group g.
                        Returns (win_hi, win_loA, win_loB) rearranged views."""
                        g0 = 4 * g * PADW
                        w_hi = win.tile([128, 6 * PADW], BF16, name="win_hi",
                                        tag="win_hi")
                        w_loA = win.tile([128, 6 * PADW], BF16, name="win_loA",
                                         tag="win_loA")
                        w_loB = win.tile([128, 6 * PADW], BF16, name="win_loB",
                                         tag="win_loB")
                        nc.sync.dma_start(
                            w_hi[:], xp_d.ap()[0:128, g0:g0 + 6 * PADW])
                        # A: lower = x_lo, upper = x_lo shifted +2 cols
                        nc.sync.dma_start(
                            w_loA[0:64, :], xp_d.ap()[128:192, g0:g0 + 6 * PADW])
                        nc.sync.dma_start(
                            w_loA[64:128, :],
                            xp_d.ap()[128:192, g0 + 2:g0 + 2 + 6 * PADW])
                        # B: lower = x_lo rows 0..3, upper = x_lo rows 2..5
                        # (only buffer rows 0..3 are ever sliced)
                        nc.sync.dma_start(
                            w_loB[0:64, 0:4 * PADW],
                            xp_d.ap()[128:192, g0:g0 + 4 * PADW])
                        nc.sync.dma_start(
                            w_loB[64:128, 0:4 * PADW],
                            xp_d.ap()[128:192,
                                      g0 + 2 * PADW:g0 + 6 * PADW])
                        return (w_hi[:].rearrange("p (r c) -> p r c", r=6),
                                w_loA[:].rearrange("p (r c) -> p r c", r=6),
                                w_loB[:].rearrange("p (r c) -> p r c", r=6))

                    def qk_stationaries(wv_hi_v, wv_loA_v, wv_loB_v, r):
                        """The 14 (stationary, K) pairs for output row r of
                        the current group, in emission order."""
                        st = []
                        for t in range(9):
                            dy, dx = t // 3, t % 3
                            st.append((wv_hi_v[:, r + dy, dx:dx + 128], t, 'hi'))
                        for s, (t1, t2) in enumerate(PAIRS_A):
                            d = t1 // 3
                            st.append((wv_loA_v[:, r + d, 0:128], s, 'lo'))
                        st.append((wv_loB_v[:, r, 1:129], 3, 'lo'))
                        st.append((wv_loA_v[0:64, r + 1, 1:129], 4, 'lo'))
                        return st

                    for g in range(32):
                        wv_hi_v, wv_loA_v, wv_loB_v = load_windows(g)
                        for r in range(4):
                            y = 4 * g + r
                            qk_ps = cps.tile([128, 384], F32, name="qk_ps",
                                             tag="qk_ps")
                            sts = qk_stationaries(wv_hi_v, wv_loA_v, wv_loB_v, r)
                            n = len(sts)
                            for i, (stat, idx, kind) in enumerate(sts):
                                w = wqk_hi if kind == 'hi' else wqk_lop
                                mov = w[:, idx * 384:(idx + 1) * 384]
                                if kind == 'lo' and idx == 4:
                                    mov = wqk_lop[0:64, idx * 384:(idx + 1) * 384]
                                nc.tensor.matmul(
                                    qk_ps[:], stat, mov,
                                    start=(i == 0), stop=(i == n - 1))
                            # deferred G (one row back) and ssq (two rows back)
                            if pend_g:
                                emit_g(*pend_g.pop(0))
                            if len(pend_ssq) > 1:
                                emit_ssq(*pend_ssq.pop(0))
                            if "nocp" in ab:
                                continue
                            qk_sb = qkp.tile([128, 384], BF16, name="qk_sb",
                                             tag="qk_sb")
                            nc.scalar.copy(qk_sb[:], qk_ps[:])
                            sq_sb = sqp_pool.tile([128, 384], BF16, name="sq_sb",
                                                  tag="sq_sb")
                            nc.scalar.square(sq_sb[:], qk_sb[:])
                            if "nog" not in ab:
                                pend_g.append((qk_sb, y))
                                pend_ssq.append((sq_sb, y))

                    if "noattn" in ab:
                        ph = qkp.tile([128, 384], F32, name="probe", tag="qk_sb")
                        nc.vector.tensor_copy(ph[:], qk_ps[:])
                        nc.scalar.dma_start(y_d.ap()[0:128, 0:384], ph[:])
                        mm_stk.close()
                        return
                    # drain deferred work
                    while pend_g:
                        emit_g(*pend_g.pop(0))
                    while pend_ssq:
                        emit_ssq(*pend_ssq.pop(0))
                    mm_stk.close()  # release conv matmul PSUM banks

                    # ---- attention finalize (tiny) ----
                    aps_stk = gps_stk.enter_context(contextlib.ExitStack())
                    aps = aps_stk.enter_context(
                        tc.tile_pool(name="aps", bufs=1, space="PSUM"))

                    ssq_sb = att.tile([1, 384], F32, name="ssq_sb")
                    nc.vector.tensor_copy(ssq_sb[:], ssq_ps[:])
                    norm = att.tile([1, 384], F32, name="norm")
                    nc.scalar.sqrt(norm[:], ssq_sb[:])
                    nc.vector.tensor_scalar_max(norm[:], norm[:], 1e-12)
                    rn = att.tile([1, 384], F32, name="rn")
                    nc.vector.reciprocal(rn[:], norm[:])
                    sk_r = att.tile([1, 192], BF16, name="sk_r")
                    nc.vector.tensor_copy(sk_r[:], rn[:, 192:384])
                    sq_f = att.tile([1, 192], F32, name="sq_f")
                    nc.vector.tensor_mul(sq_f[:], rn[:, 0:192], temp_f)

                    bck_ps = aps.tile([128, 192], F32, name="bck_ps")
                    nc.tensor.matmul(bck_ps[:], ones_r, sk_r[:],
                                     start=True, stop=True)
                    bck_sb = att.tile([128, 192], F32, name="bck_sb")
                    nc.scalar.copy(bck_sb[:], bck_ps[:])

                    sqp_hi = att.tile([128, 1], F32, name="sqp_hi")
                    sqp_lo = att.tile([64, 1], F32, name="sqp_lo")
                    nc.scalar.dma_start(sqp_hi[:, 0:1], sq_f[0:1, 0:128])
                    nc.scalar.dma_start(sqp_lo[:, 0:1], sq_f[0:1, 128:192])

                    gsc_hi = att.tile([128, 192], F32, name="gsc_hi")
                    nc.vector.tensor_mul(gsc_hi[:], g_hi[:], bck_sb[:])
                    gsc_lo = att.tile([64, 192], F32, name="gsc_lo")
                    nc.vector.tensor_mul(gsc_lo[:], g_lo[:], bck_sb[0:64, :])

                    mneg_hi = att.tile([128, 1], F32, name="mneg_hi")
                    mneg_lo = att.tile([64, 1], F32, name="mneg_lo")
                    bias_hi = att.tile([128, 1], F32, name="bias_hi")
                    bias_lo = att.tile([64, 1], F32, name="bias_lo")
                    den_hi = att.tile([128, 1], F32, name="den_hi")
                    den_lo = att.tile([64, 1], F32, name="den_lo")
                    e_hi = att.tile([128, 32], F32, name="e_hi")
                    e_lo = att.tile([64, 32], F32, name="e_lo")
                    a_hi = att.tile([128, 192], BF16, name="a_hi")
                    a_lo = att.tile([64, 192], BF16, name="a_lo")
                    nc.sync.dma_start(a_hi[:], az_d.ap())
                    nc.sync.dma_start(a_lo[:], az_d.ap()[0:64, :])

                    for h in range(HEADS):
                        if h < 4:
                            rows = slice(h * 32, (h + 1) * 32)
                            gsc, mneg, bias, den, e, a, sqv = (
                                gsc_hi, mneg_hi, bias_hi, den_hi, e_hi, a_hi,
                                sqp_hi)
                        else:
                            rows = slice((h - 4) * 32, (h - 3) * 32)
                            gsc, mneg, bias, den, e, a, sqv = (
                                gsc_lo, mneg_lo, bias_lo, den_lo, e_lo, a_lo,
                                sqp_lo)  # a slice below targets head cols
                        gs = gsc[rows, h * 32:(h + 1) * 32]
                        nc.vector.tensor_reduce(mneg[rows, :], gs, axis=AX.X,
                                                op=mybir.AluOpType.max,
                                                negate=True)
                        nc.vector.tensor_mul(bias[rows, :], mneg[rows, :],
                                             sqv[rows, :])
                        nc.scalar.activation(e[rows, :], gs, AF.Exp,
                                             bias=bias[rows, :],
                                             scale=sqv[rows, :],
                                             accum_out=den[rows, :])
                        nc.vector.reciprocal(den[rows, :], den[rows, :])
                        nc.vector.tensor_scalar_mul(
                            a[rows, h * 32:(h + 1) * 32], e[rows, :],
                            den[rows, :])

                    # C^T blocks: ct_hi = C^T[d=0:128, o]  ct_lo = C^T[d=128:192, o]
                    ct_ps_hi = aps.tile([128, 192], F32, name="ct_ps_hi")
                    ct_ps_lo = aps.tile([64, 192], F32, name="ct_ps_lo")
                    nc.tensor.matmul(ct_ps_hi[:], a_hi[:, 0:128], wpt_hi,
                                     start=True, stop=False)
                    nc.tensor.matmul(ct_ps_hi[:], a_lo[:, 0:128], wpt_lo,
                                     start=False, stop=True)
                    nc.tensor.matmul(ct_ps_lo[:], a_hi[:, 128:192], wpt_hi,
                                     start=True, stop=False)
                    nc.tensor.matmul(ct_ps_lo[:], a_lo[:, 128:192], wpt_lo,
                                     start=False, stop=True)
                    ct_sb_hi = att.tile([128, 192], BF16, name="ct_sb_hi")
                    ct_sb_lo = att.tile([64, 192], BF16, name="ct_sb_lo")
                    nc.vector.tensor_copy(ct_sb_hi[:], ct_ps_hi[:])
                    nc.vector.tensor_copy(ct_sb_lo[:], ct_ps_lo[:])

                    gps_stk.close()  # release gram + attention PSUM banks

                    # ---- compose Wy^T[i, o] = sum_d Wv[d,i,t] C^T[d,o] ----
                    # outputs: wy_hi [i_hi, t*192+o], wy_lop [i_lo pairs,
                    # slot*192+o], wy_s4 [i_lo, o] for the single tap.
                    wy_hi = att.tile([128, 9 * 192], BF16, name="wy_hi")
                    wy_lop = att.tile([128, 4 * 192], BF16, name="wy_lop")
                    wy_s4 = att.tile([64, 192], BF16, name="wy_s4")

                    cmp_stk = contextlib.ExitStack()
                    cmp = cmp_stk.enter_context(
                        tc.tile_pool(name="cmp", bufs=4, space="PSUM"))

                    def compose(dst, stat_hi, stat_lo):
                        """dst <- stat_hi^T ct_hi + stat_lo^T ct_lo"""
                        m = dst.partition_size()
                        ps = cmp.tile([128, 192], F32, name="wps", tag="wps")
                        nc.tensor.matmul(ps[0:m, :], stat_hi, ct_sb_hi[:],
                                         start=True, stop=False)
                        nc.tensor.matmul(ps[0:m, :], stat_lo, ct_sb_lo[:],
                                         start=False, stop=True)
                        nc.vector.tensor_copy(dst, ps[0:m, :])

                    for t in range(9):
                        cols = slice(t * 192, t * 192 + 128)
                        compose(wy_hi[:, t * 192:(t + 1) * 192],
                                wv_hi[:, cols], wv_lo[:, cols])
                    # packed lo slots (host-prepacked stationaries)
                    for s in range(4):
                        cols = slice(s * 128, (s + 1) * 128)
                        compose(wy_lop[:, s * 192:(s + 1) * 192],
                                wv_hi_p[:, cols], wv_lo_p[:, cols])
                    # single tap (1,1): i_lo only
                    t4 = TAP_SINGLE
                    cols = slice(t4 * 192 + 128, t4 * 192 + 192)
                    compose(wy_s4[:], wv_hi[:, cols], wv_lo[:, cols])
                    cmp_stk.close()

                    # ---- pass 2: y^T = x-window^T @ Wy, row by row ----
                    # stationary = x window slices (same 14 as pass 1),
                    # moving = wy [K, 192]; out [128 spatial, 192 ch].
                    yps = conv_stk.enter_context(
                        tc.tile_pool(name="yps", bufs=4, space="PSUM"))
                    fin = conv_stk.enter_context(tc.tile_pool(name="fin", bufs=4))
                    for g in range(32):
                        wv_hi_v, wv_loA_v, wv_loB_v = load_windows(g)
                        for r in range(4):
                            y = 4 * g + r
                            yt_ps = yps.tile([128, 192], F32, name="yt_ps",
                                             tag="yt_ps")
                            sts = qk_stationaries(wv_hi_v, wv_loA_v, wv_loB_v, r)
                            n = len(sts)
                            for i, (stat, idx, kind) in enumerate(sts):
                                if kind == 'hi':
                                    mov = wy_hi[:, idx * 192:(idx + 1) * 192]
                                elif idx == 4:
                                    mov = wy_s4[:]
                                else:
                                    mov = wy_lop[:, idx * 192:(idx + 1) * 192]
                                nc.tensor.matmul(
                                    yt_ps[:], stat, mov,
                                    start=(i == 0), stop=(i == n - 1))
                            yt_sb = fin.tile([128, 192], F32, name="yt_sb",
                                             tag="yt_sb")
                            if y % 2 == 0:
                                nc.scalar.copy(yt_sb[:], yt_ps[:])
                            else:
                                nc.vector.tensor_copy(yt_sb[:], yt_ps[:])
                            nc.scalar.dma_start(
                                y_d.ap()[y * 128:(y + 1) * 128, :], yt_sb[:])

            if R == 1:
                body(0)
            else:
                with tc.For_i(0, R, 1) as it:
                    body(it)

    # The neuron NEFF cache can collide across kernels with identical
    # input/output signatures; encode a BIR-content hash into a dummy
    # input's shape so every distinct build gets a distinct HLO.
    import zlib
    h = zlib.crc32(nc.to_json_bytes()) % 997 + 1
    nc.dram_tensor("cachebust", [1, h], F32, kind="ExternalInput")
    return nc


class PjrtRunner:
    """Build the jitted SPMD executable once; allow repeated timed runs."""

    def __init__(self, nc, n_cores=8):
        from jax.sharding import Mesh, PartitionSpec
        from jax.experimental.shard_map import shard_map
        bass2jax.install_neuronx_cc_hook()
        self.nc = nc
        self.n_cores = n_cores
        partition_name = (nc.partition_id_tensor.name
                          if nc.partition_id_tensor else None)
        in_names, out_names, out_avals = [], [], []
        for alloc in nc.m.functions[0].allocations:
            if not isinstance(alloc, mybir.MemoryLocationSet):
                continue
            name = alloc.memorylocations[0].name
            if alloc.kind == "ExternalInput":
                if name != partition_name:
                    in_names.append(name)
            elif alloc.kind == "ExternalOutput":
                out_names.append(name)
                out_avals.append(jax.core.ShapedArray(
                    tuple(alloc.tensor_shape), mybir.dt.np(alloc.dtype)))
        self.in_names, self.out_names, self.out_avals = (
            in_names, out_names, out_avals)
        n_params = len(in_names)
        all_in_names = list(in_names) + list(out_names)
        if partition_name is not None:
            all_in_names.append(partition_name)

        def _body(*args):
            operands = list(args)
            if partition_name is not None:
                operands.append(bass2jax.partition_id_tensor())
            outs = bass2jax._bass_exec_p.bind(
                *operands,
                out_avals=tuple(out_avals),
                in_names=tuple(all_in_names),
                out_names=tuple(out_names),
                lowering_input_output_aliases=(),
                sim_require_finite=False,
                sim_require_nnan=False,
                nc=nc,
            )
            return tuple(outs)

        devices = jax.devices()[:n_cores]
        self.mesh = Mesh(np.asarray(devices), ("core",))
        in_specs = (PartitionSpec("core"),) * (n_params + len(out_names))
        out_specs = (PartitionSpec("core"),) * len(out_names)
        self.sharded = jax.jit(shard_map(
            _body, mesh=self.mesh, in_specs=in_specs, out_specs=out_specs,
            check_rep=False))

    def prepare(self, in_maps):
        n_cores = self.n_cores
        shapes = {}
        for alloc in self.nc.m.functions[0].allocations:
            if (isinstance(alloc, mybir.MemoryLocationSet)
                    and alloc.kind == "ExternalInput"):
                shapes[alloc.memorylocations[0].name] = (
                    tuple(alloc.tensor_shape), mybir.dt.np(alloc.dtype))
        def get(m, name):
            if name in m:
                return np.ascontiguousarray(np.asarray(m[name]))
            shp, dt = shapes[name]
            return np.zeros(shp, dt)
        per_core = [[get(m, name) for name in self.in_names] for m in in_maps]
        concat_in = [np.concatenate([per_core[c][i] for c in range(n_cores)],
                                    axis=0)
                     for i in range(len(self.in_names))]
        concat_zeros = [np.zeros((n_cores * a.shape[0], *a.shape[1:]), a.dtype)
                        for a in self.out_avals]
        self.dev_in = [jax.device_put(a) for a in concat_in]
        self.dev_zeros = [jax.device_put(a) for a in concat_zeros]

    def run(self):
        outs = self.sharded(*self.dev_in, *self.dev_zeros)
        jax.block_until_ready(outs)
        return outs

    def results(self, outs):
        n_cores = self.n_cores
        return [
            {name: np.asarray(outs[i]).reshape(
                n_cores, *self.out_avals[i].shape)[c]
             for i, name in enumerate(self.out_names)}
            for c in range(n_cores)
        ]


_RUNNERS = {}


def _get_runner(R=1):
    if R not in _RUNNERS:
        _RUNNERS[R] = PjrtRunner(build_nc(R), B)
    return _RUNNERS[R]


def _host_prep(x, W_qkv, W_dw, W_proj, temperature):
    from ml_dtypes import bfloat16
    x = np.asarray(x, np.float32)
    W_qkv = np.asarray(W_qkv, np.float64)
    W_dw = np.asarray(W_dw, np.float64)
    W_proj = np.asarray(W_proj, np.float32)
    temperature = np.asarray(temperature, np.float32)

    # fused conv weights: W_f[o,i,dy,dx] = sum_m W_dw[o,m,dy,dx] W_qkv[m,i]
    wd = W_dw.transpose(0, 2, 3, 1).reshape(576 * 9, 576)  # [o*dy*dx, m]
    wf = (wd @ W_qkv[:, :, 0, 0]).reshape(576, 3, 3, DIM)  # [o,dy,dx,i]
    wfi = wf.transpose(3, 1, 2, 0).reshape(DIM, 9, 576)  # [i, t, o]

    # qk part (o in 0:384): moving layout [ci, t*384+co]
    wqk = wfi[:, :, 0:384].astype(bfloat16)           # [i, t, 384]
    wqk_hi = np.ascontiguousarray(wqk[0:128].reshape(128, 9 * 384))
    wqk_lop = np.zeros((128, 5 * 384), bfloat16)
    for s, (t1, t2) in enumerate(PAIRS_A + [PAIR_B]):
        wqk_lop[0:64, s * 384:(s + 1) * 384] = wqk[128:192, t1]
        wqk_lop[64:128, s * 384:(s + 1) * 384] = wqk[128:192, t2]
    wqk_lop[0:64, 4 * 384:5 * 384] = wqk[128:192, TAP_SINGLE]

    # v part for the compose step: Wv[d, i, t] = wf[384+d, dy, dx, i]
    # layout [d, t*192 + i]
    wv = wf[384:576].transpose(0, 1, 2, 3).reshape(192, 9, 192)  # [d, t, i]
    wv = wv.astype(bfloat16)
    wv_hi = np.ascontiguousarray(wv[0:128].reshape(128, 9 * 192))
    wv_lo = np.ascontiguousarray(wv[128:192].reshape(64, 9 * 192))
    # packed-lo compose stationaries: [d, slot*128 + (i_lo(t1) | i_lo(t2))]
    wv_hi_p = np.zeros((128, 4 * 128), bfloat16)
    wv_lo_p = np.zeros((64, 4 * 128), bfloat16)
    for s, (t1, t2) in enumerate(PAIRS_A + [PAIR_B]):
        wv_hi_p[:, s * 128:s * 128 + 64] = wv[0:128, t1, 128:192]
        wv_hi_p[:, s * 128 + 64:(s + 1) * 128] = wv[0:128, t2, 128:192]
        wv_lo_p[:, s * 128:s * 128 + 64] = wv[128:192, t1, 128:192]
        wv_lo_p[:, s * 128 + 64:(s + 1) * 128] = wv[128:192, t2, 128:192]

    wpt = W_proj[:, :, 0, 0].T.astype(bfloat16)  # [c_in, o]
    consts_hi = np.concatenate(
        [wpt[0:128], np.ones((128, 1), bfloat16)], axis=1)
    consts_lo = np.ascontiguousarray(wpt[128:192])
    temp_ext = np.repeat(temperature.reshape(HEADS), 32).astype(np.float32)
    consts_row = np.concatenate(
        [temp_ext, np.ones(128, np.float32)]).reshape(1, 320)

    in_maps = []
    for b in range(B):
        xp = np.zeros((DIM, XCOLS), bfloat16)
        xpv = xp[:, :PADW * PADW].reshape(DIM, PADW, PADW)
        xpv[:, 1:129, 1:129] = x[b].astype(bfloat16)
        in_maps.append({
            "xp": xp,
            "wqk_hi": wqk_hi, "wqk_lop": wqk_lop,
            "wv_hi": wv_hi, "wv_lo": wv_lo,
            "wv_hi_p": wv_hi_p, "wv_lo_p": wv_lo_p,
            "consts_hi": consts_hi, "consts_lo": consts_lo,
            "consts_row": consts_row,
            "ones_row": np.ones((1, 128), bfloat16),
            "azero": np.zeros((128, 192), bfloat16),
        })
    return in_maps


def kernel(x, W_qkv, W_dw, W_proj, temperature):
    in_maps = _host_prep(x, W_qkv, W_dw, W_proj, temperature)
    r = _get_runner(1)
    r.prepare(in_maps)
    res = r.results(r.run())
    out = np.stack([
        np.ascontiguousarray(
            res[b]["yt"].reshape(128, 128, DIM).transpose(2, 0, 1))
        for b in range(B)])
    return out.astype(np.float32)


def measure_hw_time_ns(inputs, R=17, n_pairs=10):
    """Paired interleaved timing of R=1 vs R=R NEFFs; returns est ns/iter."""
    in_maps = _host_prep(**inputs)
    r1 = _get_runner(1)
    rR = _get_runner(R)
    r1.prepare(in_maps)
    rR.prepare(in_maps)
    r1.run(); rR.run()
    d1, dR = [], []
    for _ in range(n_pairs):
        t0 = time.perf_counter(); r1.run(); d1.append(time.perf_counter() - t0)
        t0 = time.perf_counter(); rR.run(); dR.append(time.perf_counter() - t0)
    d1 = np.array(d1); dR = np.array(dR)
    est_med = (np.median(dR) - np.median(d1)) / (R - 1) * 1e9
    est_min = (dR.min() - d1.min()) / (R - 1) * 1e9
    return est_med, est_min
